# revision 4
# baseline (speedup 1.0000x reference)
"""BiLSTM language model kernel for Trainium2 (8 NeuronCores), v3.

Sharding: data-parallel over batch (B=32 -> 4 per core). Each core runs the
full bidirectional LSTM scan for its batch slice and the full-vocab output
projection for its tokens locally (no collectives).

v3 design (vs v2's cache-everything two-phase projection):
  - scan as TWO independent interleaved chains (fwd / rev), each [*, 4]
    wide. The per-step cross-engine latency (MM -> ACT -> DVE -> ACT ->
    DVE) of one chain hides under the other; dead step t=63 dropped.
  - projection emits per vocab group: PE matmul -> exp (+ per-group
    f32 accum into `partials`) -> DVE bit-log -> DMA out, with NO
    dependency on the softmax sum: the DMA'd logits are UNNORMALIZED
    (out = l + sawtooth). The device also outputs `partials` ([128,66]
    f32 exp-sums); the host computes log S per token and subtracts it
    during unshard/assemble. This removes v2's A-phase fill / B-phase
    drain serialization and the 13MB SBUF exp cache; output DMA streams
    through the whole projection.
  - ACT/DVE exp split: ~2/3 of groups use ACT Exp (true exp, bf16 cache,
    free accum); every third group uses a DVE Schraudolph bit-exp:
    bits16 = round(l*128*log2e + K16) as int16 == bf16 bits of ~e^l.
    Its group sum comes from a kappa-corrected Pool-engine accumulate
    pass over the bf16 cache (Pool is otherwise idle).
  - bit-log decode (both kinds): ob = u16(cache)*(ln2/128) - OFF with
    OFF = (127 - 0.0431)*ln2, a constant -> tensor_scalar at 4x. The
    bit-log + DMA of group g are emitted in round g+1 so the DVE queue
    never head-of-line blocks behind ACT's exp.
  - tanh-only scan (sigmoid via tanh(x/2), hidden stored as 2h, cell as
    s=2C) with host-prepped folded weights, as in v2.

DVE 2-input ops need both SBUF inputs at the same base partition; gates land
as tanh_f@0, tanh_i@32, tanh_o@64 with ct parked @32 and tanh(C) @64.
"""

import numpy as np
from contextlib import ExitStack

from concourse import inst_simplify

import concourse.bass as bass
import concourse.mybir as mybir
import concourse.tile as tile
from concourse import bacc
from concourse.masks import make_identity

F32 = mybir.dt.float32
BF16 = mybir.dt.bfloat16
U16 = mybir.dt.uint16
I16 = mybir.dt.int16
I32 = mybir.dt.int32
AF = mybir.ActivationFunctionType
ALU = mybir.AluOpType

S = 64          # sequence length
B = 32          # full batch
V = 50257       # vocab
HID = 16
EMB = 32
NCORES = 8
BL = B // NCORES          # batch per core = 4
T = S * BL                # tokens per core = 256
KC = EMB + HID            # 48
GP = 112                  # gate rows: tanh_f@0, tanh_i@32, tanh_o@64, tanh_c@96
REV = (S + 1) * BL        # column offset of reverse region in comb = 260
GW = 1536                 # vocab columns per group (3 PSUM banks)
BAND = 26112              # vocab columns in band 0 (= 17 groups)

LN2 = float(np.log(2.0))
# ob = u16(cache)*(LN2/128) - OFF recovers ~l from the bf16 bit pattern of
# e^l; OFF centers the piecewise-linear log sawtooth (+-0.030 abs).
OFF = (127.0 - 0.0431) * LN2
# DVE bit-exp: bits16 = round(l*SIG16 + K16) == bf16 bits of ~e^l, chosen
# so the bit-log above decodes it back to exactly l (+-i16 rounding).
SIG16 = 128.0 / LN2
K16 = (127.0 - 0.0431) * 128.0
# mean of bitcast(bits16)/e^l over a uniform mantissa: correction for the
# Pool accumulate pass so DVE-group exp sums are unbiased.
KAPPA = 0.9900458566069128

# groups: (band, local_start, width, global_start, out_width)
GROUPS = []
for g in range(17):
    GROUPS.append((0, g * GW, GW, g * GW, GW))
for g in range(15):
    GROUPS.append((1, g * GW, GW, BAND + g * GW, GW))
# last group padded to even width 1106; the phantom zero-weight column adds
# exp(0)=1 to each token's sum (subtracted on host) and is not DMA'd out.
GROUPS.append((1, 15 * GW, 1106, BAND + 15 * GW, 1105))
NG = len(GROUPS)          # 33

# groups whose exp runs on DVE (Schraudolph) instead of ACT; ~1/3, spread
# evenly, never the padded last group.
DVE_G = frozenset(g for g in range(NG) if g % 3 == 1)


def build_nc(compile=True):
    nc = bacc.Bacc("TRN2", target_bir_lowering=False, debug=False)

    # ---------------- DRAM I/O ----------------
    d_emb = nc.dram_tensor("emb_table", [V, EMB], F32, kind="ExternalInput")
    d_idx = nc.dram_tensor("idx", [T, 1], I32, kind="ExternalInput")
    d_wcomb = nc.dram_tensor("w_combT", [KC, GP], BF16, kind="ExternalInput")
    d_bcell = nc.dram_tensor("b_cell", [GP, 1], F32, kind="ExternalInput")
    d_h0 = nc.dram_tensor("h0", [HID, BL], BF16, kind="ExternalInput")
    d_s0 = nc.dram_tensor("s0", [HID, BL], F32, kind="ExternalInput")
    d_wlo = nc.dram_tensor("w_lo", [33, BAND], BF16, kind="ExternalInput")
    d_whi = nc.dram_tensor("w_hi", [33, BAND], BF16, kind="ExternalInput")
    d_out = nc.dram_tensor("out", [T, V], BF16, kind="ExternalOutput")
    d_part = nc.dram_tensor("partials", [128, 2 * NG], F32, kind="ExternalOutput")

    with tile.TileContext(nc) as tc, ExitStack() as ctx:
        singles = ctx.enter_context(tc.tile_pool(name="singles", bufs=1))
        # projection PSUM first so it gets banks disjoint from the scan pool
        psP = ctx.enter_context(tc.tile_pool(name="psP", bufs=2, space="PSUM"))
        cachep = ctx.enter_context(tc.tile_pool(name="cache", bufs=3))
        obp = ctx.enter_context(tc.tile_pool(name="ob", bufs=4))

        # gather inputs first, in four 64-token segments ordered so the
        # bidirectional scan can start after the first two: seg 0 feeds the
        # fwd chain's early steps, seg 3 the rev chain's. Each 64-row
        # indirect gather costs ~6us of SWDGE descriptor generation, so the
        # later segments pipeline under the early scan steps.
        SEGS = [0, 3, 1, 2]                      # segment k = tokens 64k..64k+64
        gat = ctx.enter_context(tc.tile_pool(name="pgather", bufs=8))
        idx_sbs, embgs = {}, {}
        for seg in SEGS:
            idx_sb = gat.tile([64, 1], I32, tag=f"idx{seg}", name=f"idx{seg}")
            nc.sync.dma_start(idx_sb, d_idx.ap()[seg * 64 : (seg + 1) * 64, :])
            idx_sbs[seg] = idx_sb
        for seg in SEGS:
            embg = gat.tile([64, EMB], F32, tag=f"embg{seg}", name=f"embg{seg}")
            nc.gpsimd.indirect_dma_start(
                out=embg,
                out_offset=None,
                in_=d_emb.ap(),
                in_offset=bass.IndirectOffsetOnAxis(ap=idx_sbs[seg][:, :1], axis=0),
            )
            embgs[seg] = embg

        ident = singles.tile([128, 128], F32)
        make_identity(nc, ident)

        w_combT = singles.tile([KC, GP], BF16)
        nc.sync.dma_start(w_combT, d_wcomb.ap())
        b_cell = singles.tile([GP, 1], F32)
        nc.sync.dma_start(b_cell, d_bcell.ap())

        # comb: [48, 520] bf16; cols 0..260 fwd blocks 0..64, cols 260..520
        # rev blocks 0..64. rows 0-31 = x (embT), rows 32-47 = 2h state.
        comb = singles.tile([KC, 2 * REV], BF16)
        # concat features, bf16: rows 0-15 lefts, 16-31 rights, 32 ones; the
        # same three bands duplicated at rows 64-96 for band-1 matmuls.
        ca2 = singles.tile([97, T], BF16)
        nc.vector.memset(ca2[32:33, :], 1.0)
        nc.vector.memset(ca2[96:97, :], 1.0)

        # s = 2C state [16, 4] f32 per chain
        s_f = singles.tile([HID, BL], F32)
        s_r = singles.tile([HID, BL], F32)
        nc.sync.dma_start(s_f, d_s0.ap())
        nc.sync.dma_start(s_r, d_s0.ap())
        nc.sync.dma_start(comb[EMB:KC, 0:BL], d_h0.ap())            # fwd block 0
        nc.sync.dma_start(comb[EMB:KC, 2 * REV - BL : 2 * REV], d_h0.ap())  # rev 64

        # two-band projection weights: rows 0:33 = vocab [0, 26112),
        # rows 64:97 = vocab [26112, 50257) zero-padded.
        w_sb = singles.tile([97, BAND], BF16)
        nc.scalar.dma_start(w_sb[0:33, :], d_wlo.ap())
        nc.scalar.dma_start(w_sb[64:97, :], d_whi.ap())

        # per-group exp sums; col = tile*NG + g
        partials = singles.tile([128, 2 * NG], F32)
        # sink for the Pool accumulate pass's mandatory tensor output
        junk = singles.tile([128, GW], BF16)

        # ---------------- embedding transpose via PE, per segment ----------------
        # x parts of comb: fwd block t = token t; rev block m+1 = token m
        with tc.tile_pool(name="ps_misc", bufs=2, space="PSUM") as psm:
            for seg in SEGS:
                ps_tr = psm.tile([EMB, 64], F32)
                nc.tensor.transpose(ps_tr, embgs[seg], ident[0:64, 0:64])
                c0, c1 = seg * 64, (seg + 1) * 64
                nc.vector.tensor_copy(comb[0:EMB, c0:c1], ps_tr)
                nc.vector.tensor_copy(
                    comb[0:EMB, REV + BL + c0 : REV + BL + c1], comb[0:EMB, c0:c1]
                )

        # ---------------- scan machinery ----------------
        ssb = ctx.enter_context(tc.tile_pool(name="scan_sb", bufs=10))
        sps = ctx.enter_context(tc.tile_pool(name="scan_ps", bufs=2, space="PSUM"))

        s_st = {0: s_f, 1: s_r}

        def scan_mm(t, d):
            in_col = BL * t if d == 0 else REV + BL * (S - t)
            g_ps = sps.tile([GP, BL], F32, tag=f"g{d}", name=f"g{d}_{t}")
            nc.tensor.matmul(
                g_ps, w_combT, comb[:, in_col : in_col + BL], start=True, stop=True
            )
            return g_ps

        def scan_act(t, d, g_ps):
            # tg rows: tanh(zf/2)@0, tanh(zi/2)@32, tanh(zo/2)@64
            tg = ssb.tile([96, BL], F32, tag=f"tg{d}")
            nc.scalar.activation(tg, g_ps[0:96, :], AF.Tanh, bias=b_cell[0:96, :])
            ct = ssb.tile([48, BL], F32, tag=f"ct{d}")   # tanh(z_C) @ 32
            nc.scalar.activation(
                ct[32:48, :], g_ps[96:GP, :], AF.Tanh, bias=b_cell[96:GP, :]
            )
            return tg, ct

        def scan_dve(t, d, tg, ct):
            # s_new = 2C_new = 0.5*(tf+1)*s + (ti+1)*ct
            p = ssb.tile([48, BL], F32, tag=f"p{d}")
            nc.vector.scalar_tensor_tensor(
                p[32:48, :], tg[0:HID, :], 1.0, s_st[d][:, :], ALU.add, ALU.mult
            )
            q = ssb.tile([48, BL], F32, tag=f"q{d}")
            nc.vector.scalar_tensor_tensor(
                q[32:48, :], tg[32 : 32 + HID, :], 1.0, ct[32:48, :],
                ALU.add, ALU.mult,
            )
            nc.vector.scalar_tensor_tensor(
                s_st[d][:, :], p[32:48, :], 0.5, q[32:48, :], ALU.mult, ALU.add
            )

        def scan_th(t, d):
            th = ssb.tile([80, BL], F32, tag=f"th{d}")   # tanh(C_new) @ 64
            nc.scalar.activation(th[64:80, :], s_st[d][:, :], AF.Tanh, scale=0.5)
            return th

        def scan_h(t, d, tg, th):
            # 2h = (to+1)*tanh(C_new) -> fwd block t+1 / rev block 63-t
            out_col = BL * (t + 1) if d == 0 else REV + BL * (S - 1 - t)
            nc.vector.scalar_tensor_tensor(
                comb[EMB:KC, out_col : out_col + BL],
                tg[64 : 64 + HID, :], 1.0, th[64:80, :], ALU.add, ALU.mult,
            )

        # ---------------- projection machinery ----------------
        def ca2_copy(tile_idx):
            """Fill ca2 cols for one 128-token tile (32 s-values)."""
            c0, c1 = tile_idx * 128, (tile_idx + 1) * 128
            s0c = tile_idx * 32
            n = 128
            # lefts: fwd h blocks s0c.. (block s holds lefts[s])
            nc.vector.tensor_copy(
                ca2[0:HID, c0:c1], comb[EMB:KC, BL * s0c : BL * s0c + n]
            )
            # rights: rev blocks s0c+1.. (block m holds flipped[m]; block 64=h0)
            nc.sync.dma_start(
                ca2[HID : 2 * HID, c0:c1],
                comb[EMB:KC, REV + BL * (s0c + 1) : REV + BL * (s0c + 1) + n],
            )
            # duplicate both bands at partitions 64:96 (input must start at a
            # 32-aligned partition, so one copy of rows 0:32)
            nc.vector.tensor_copy(ca2[64:96, c0:c1], ca2[0 : 2 * HID, c0:c1])

        def proj_mm_exp(tile_idx, gi):
            band, ls, cw, _, _ = GROUPS[gi]
            r0 = 64 * band
            lhs = ca2[r0 : r0 + 33, tile_idx * 128 : (tile_idx + 1) * 128]
            ps = psP.tile([128, GW], F32, tag="ps", name=f"ps{tile_idx}_{gi}")
            for j0 in range(0, cw, 512):
                jw = min(512, cw - j0)
                nc.tensor.matmul(
                    ps[:, j0 : j0 + jw],
                    lhs,
                    w_sb[r0 : r0 + 33, ls + j0 : ls + j0 + jw],
                    start=True,
                    stop=True,
                )
            eb = cachep.tile([128, GW], BF16, tag="cache", name=f"eb{tile_idx}_{gi}")
            pcol = tile_idx * NG + gi
            if gi in DVE_G:
                # Schraudolph bit-exp on DVE: int16 bits == bf16(~e^l)
                nc.vector.tensor_scalar(
                    eb[:, :cw].bitcast(I16), ps[:, :cw],
                    SIG16, K16, ALU.mult, ALU.add,
                )
                # unbiased group sum on the otherwise-idle Pool engine
                nc.gpsimd.tensor_scalar(
                    junk[:, :cw], eb[:, :cw], KAPPA, None, ALU.mult,
                    accum_out=partials[:, pcol : pcol + 1],
                )
            else:
                nc.scalar.activation(
                    eb[:, :cw], ps[:, :cw], AF.Exp,
                    accum_out=partials[:, pcol : pcol + 1],
                )
            return eb

        def proj_blog(tile_idx, gi, eb):
            _, _, cw, gs, ow = GROUPS[gi]
            ob = obp.tile([128, GW], BF16, tag="ob", name=f"ob{tile_idx}_{gi}")
            nc.vector.tensor_scalar(
                ob[:, :cw], eb[:, :cw].bitcast(U16),
                LN2 / 128.0, OFF, ALU.mult, ALU.subtract,
            )
            nc.sync.dma_start(
                d_out.ap()[tile_idx * 128 : (tile_idx + 1) * 128, gs : gs + ow],
                ob[:, :ow],
            )

        # ---------------- schedule ----------------
        # scan steps 0..62 (step 63's outputs are dead), chains interleaved
        for t in range(S - 1):
            g_f = scan_mm(t, 0)
            g_r = scan_mm(t, 1)
            tg_f, ct_f = scan_act(t, 0, g_f)
            tg_r, ct_r = scan_act(t, 1, g_r)
            scan_dve(t, 0, tg_f, ct_f)
            scan_dve(t, 1, tg_r, ct_r)
            th_f = scan_th(t, 0)
            th_r = scan_th(t, 1)
            scan_h(t, 0, tg_f, th_f)
            scan_h(t, 1, tg_r, th_r)

        ca2_copy(0)
        ca2_copy(1)

        # projection: bit-log + DMA of group g ride in round g+1 so the DVE
        # queue never parks behind ACT's exp
        prev = None
        for tile_idx in (0, 1):
            for gi in range(NG):
                eb = proj_mm_exp(tile_idx, gi)
                if prev is not None:
                    proj_blog(*prev)
                prev = (tile_idx, gi, eb)
        proj_blog(*prev)
        nc.sync.dma_start(d_part.ap(), partials)

    if compile:
        _compile_with_ldw_dedup(nc)
    return nc


def _ldw_key(inst):
    a = inst.ins[0]
    return (
        getattr(a, "memref", None),
        getattr(a, "offset", None),
        str(getattr(a, "ap", None)),
        str(getattr(a, "dtype", None)),
        str(inst.perf_mode),
        str(inst.is_transpose),
        str(inst.tile_position),
    )


def _dedup_ldweights(nc):
    """Drop LDWEIGHTS instructions whose weights are already loaded (same AP
    as the previous PE weight load, with no other weight-changing PE
    instruction in between). Same-weight matmuls then issue back-to-back and
    pipeline on the PE instead of serializing on redundant reloads."""
    fn = nc.m.functions[0]
    n_drop = 0
    for bb in fn.blocks:
        out = []
        last_key = None
        carry = []
        for inst in bb.instructions:
            nm = inst.__class__.__name__
            if nm == "InstLdweights":
                si = inst.sync_info
                has_upd = bool(si and si.on_update)
                key = _ldw_key(inst)
                if key == last_key and not has_upd:
                    if si and si.on_wait:
                        carry.extend(si.on_wait)
                    n_drop += 1
                    continue
                last_key = key
            elif nm == "InstMatmult":
                if carry:
                    si = inst.sync_info
                    w = list(si.on_wait) if si and si.on_wait else []
                    si.on_wait = carry + w
                    inst.sync_info = si
                    carry = []
                # self-loading matmuls (f32 / transposes) clobber the array
                if inst.is_transpose or str(
                    getattr(inst.ins[0], "dtype", "")
                ) in ("dt.float32", "dt.float32r"):
                    last_key = None
            out.append(inst)
        assert not carry, "dropped-LDW waits with no following matmul"
        bb.instructions = out
    return n_drop


def _compile_with_ldw_dedup(nc):
    """bacc.Bacc.compile() with an LDWEIGHTS-dedup pass inserted right after
    move_matmul_waits_to_ldweights."""
    nc.insert_bir_kernel_barrier_sem_inc()
    nc.move_matmul_waits_to_ldweights()
    _dedup_ldweights(nc)
    nc.generate_event_semaphores()
    nc.remove_dead_instructions_after_branch()
    nc.validate_blocks()
    nc.dce_regs()
    nc.thread_jumps()
    nc.remove_dead_blocks()
    nc.remove_dead_allocations()
    nc.verify_switch_hints()
    nc.alloc_regs()
    inst_simplify.simplify(nc)
    nc.fuse_regops()
    nc.fuse_blocks()
    nc.replace_nops_with_events()
    for engine in nc.engines:
        nc.fuse_nops(engine)
    nc.remove_dead_nops()
    nc.remove_dangling_data()
    nc.generate_event_semaphores()
    nc.insert_library_loads()
    nc.insert_act_table_loads()
    nc.insert_hostgen_rebases()
    nc.codegen_inst_isa_subclasses()


def host_prep(inputs, ncores=NCORES):
    """Build the per-core input maps from the full problem inputs."""
    import ml_dtypes

    emb = np.ascontiguousarray(np.asarray(inputs["embedding"], dtype=np.float32))
    ib = np.asarray(inputs["input_batch"]).astype(np.int32)          # [S, B]
    W = [np.asarray(inputs[k], dtype=np.float32) for k in ("W_f", "W_i", "W_o", "W_C")]
    b = [np.asarray(inputs[k], dtype=np.float32) for k in ("b_f", "b_i", "b_o", "b_C")]
    W_ho = np.asarray(inputs["W_ho"], dtype=np.float32)
    b_ho = np.asarray(inputs["b_ho"], dtype=np.float32)
    h0 = np.asarray(inputs["initial_hidden"], dtype=np.float32)      # [1, HID]
    c0i = np.asarray(inputs["initial_C"], dtype=np.float32)

    # gate rows: f@0, i@32, o@64, c@96. sigmoid gates become tanh(z/2): W,b
    # halved; all h-columns halved again because the stored hidden is 2h.
    Wc = np.zeros((GP, KC), dtype=np.float32)
    bc = np.zeros((GP, 1), dtype=np.float32)
    for gi, (Wg, bg) in enumerate(zip(W, b)):
        gate_scale = 0.5 if gi < 3 else 1.0
        Wrow = Wg * gate_scale
        Wrow = np.concatenate([Wrow[:, :EMB], Wrow[:, EMB:] * 0.5], axis=1)
        Wc[32 * gi : 32 * gi + HID] = Wrow
        bc[32 * gi : 32 * gi + HID, 0] = bg * gate_scale
    w_combT = np.ascontiguousarray(Wc.T.astype(ml_dtypes.bfloat16))  # [48, 112]

    # projection: rows 0:32 = W_ho.T * 0.5 (features are 2h), row 32 = b_ho
    w_full = np.empty((33, V), dtype=np.float32)
    w_full[0:EMB] = W_ho.T * 0.5
    w_full[EMB] = b_ho
    w_full = w_full.astype(ml_dtypes.bfloat16)
    w_lo = np.ascontiguousarray(w_full[:, :BAND])
    w_hi = np.zeros((33, BAND), dtype=ml_dtypes.bfloat16)
    w_hi[:, : V - BAND] = w_full[:, BAND:]

    h0T = np.ascontiguousarray(
        np.broadcast_to(2.0 * h0.T, (HID, BL))
    ).astype(ml_dtypes.bfloat16)
    s0T = np.ascontiguousarray(
        np.broadcast_to(2.0 * c0i.T, (HID, BL))
    ).astype(np.float32)

    bl = B // ncores
    in_maps = []
    for c in range(ncores):
        idx = np.ascontiguousarray(
            ib[:, c * bl : (c + 1) * bl].reshape(T, 1)
        )  # token t = s*BL + b
        in_maps.append(
            {
                "emb_table": emb,
                "idx": idx,
                "w_combT": w_combT,
                "b_cell": np.ascontiguousarray(bc),
                "h0": h0T,
                "s0": s0T,
                "w_lo": w_lo,
                "w_hi": w_hi,
            }
        )
    return in_maps


def assemble_output(raw_outs):
    """Per core: [T, V] bf16 unnormalized bit-log logits + [128, 2*NG] f32
    exp partials -> [S, B, V] f32 log-probs."""
    outs = []
    for r in raw_outs:
        ob = np.asarray(r["out"]).astype(np.float32)          # [256, V] ~ l
        part = np.asarray(r["partials"]).astype(np.float64)   # [128, 2*NG]
        s0 = part[:, :NG].sum(axis=1) - 1.0   # phantom pad column adds 1
        s1 = part[:, NG:].sum(axis=1) - 1.0
        logS = np.log(np.concatenate([s0, s1])).astype(np.float32)  # [256]
        outs.append((ob - logS[:, None]).reshape(S, BL, V))
    return np.concatenate(outs, axis=1)


_NC_CACHE = {}


def kernel(**inputs):
    from concourse.bass_utils import run_bass_kernel_spmd

    if "full" not in _NC_CACHE:
        _NC_CACHE["full"] = build_nc()
    nc = _NC_CACHE["full"]
    in_maps = host_prep(inputs)
    res = run_bass_kernel_spmd(nc, in_maps, core_ids=list(range(NCORES)))
    return assemble_output(res.results)
